# revision 53
# baseline (speedup 1.0000x reference)
"""Trainium2 Bass kernel for nn_CategorySpecificInitNet (moe_routing).

kernel(**inputs) takes the FULL unsharded inputs (keys as in
reference.setup_inputs()) and returns the FULL [B, 128] float32 output.

Strategy — expert-parallel + fp8 DoubleRow compensated matmuls:
  - rows are dispatched to cores by category: core k's MAIN block is 4096
    rows (8 tiles x 512) of category k, so the main block runs ONE decoder
    with no routing/masking; categories with more than 4096 rows spill
    into small per-core TAIL tiles (decoder weight set "B");
  - the encoder's third linear layer is constant-folded into each
    decoder's first layer on the host (W_f = We3 @ Wd1_k, exact algebra);
  - every GEMM runs as fp8e4 (e4m3) matmuls in DoubleRow perf mode
    (2 contraction rows per PE cell), with a 3-slot error-compensated
    split: w = w_hi + w_lo, x = x_hi + x_lo (each e4m3), computing
    w_hi*x_hi + w_lo*x_hi + w_hi*x_lo. Per 256 contraction rows this is
    3 DR matmuls instead of 2 plain bf16 matmuls = 0.75x PE cycles at
    ~bf16-grade accuracy (the dropped w_lo*x_lo term is ~2^-8 relative);
  - power-of-2 scaling keeps every e4m3 quantization point in the
    format's sweet spot: weights scaled to sigma~0.35, features to ~0.5,
    and the psum scale grows geometrically layer to layer (all quantize
    points stay under e4m3's +-240 ceiling), reset by mid-net rescales at
    the L2 and d2 outputs; the final fp32 psum is unwound by a single
    pow2 scale in the copy-out;
  - activation hi/lo streams are produced straight from PSUM:
      A/D blocks (L1, d1 outs):  ACT hh=e4m3(relu(p)),
                                 DVE hl=e4m3(relu(p)-hh)  [scalar_tensor_tensor]
      B/C blocks (L2, d2 outs):  ACT hh=e4m3(relu(p)*s), DVE h16=fp16(relu(p)*s),
                                 GPSIMD hl=e4m3(h16-hh)   [rescale rides free]
    one op per engine per block, fitting under the PE time per tile;
  - psum tiles are fused chunk-pairs [128, 2, 512] (2 banks) so one
    ACT/DVE op covers a DoubleRow pair and the activation tiles are
    directly the [128, 2, N] DR moving operand;
  - software pipeline per step t: [L1(t)][d2(t-2)][d3(t-3)][L2(t)][d1(t-1)]
    hides the ~1.4/2.7us hh/hl latency chains behind matmuls of other
    tiles; 2+2 psum pair-pool slots (8 banks) exactly cover the rotation;
  - a warm-up chain of dummy matmuls covers the tensor engine p-state
    ramp while the first weight/feature DMAs stream in (all on the sync
    queue, interleaved pair-by-pair so L1 tile 0 streams behind them).

Measured end-to-end error vs the fp32 reference: ~4.6e-3 max-rel (gate 2e-2).
"""
import sys

for _p in ("/opt/trn_rl_repo",):
    if _p not in sys.path:
        sys.path.append(_p)

import numpy as np
import ml_dtypes

import concourse.bass as bass
import concourse.bacc as bacc
import concourse.mybir as mybir
import concourse.tile as tile
from concourse import bass_utils

F32 = mybir.dt.float32
BF = mybir.dt.bfloat16
FP16 = mybir.dt.float16
FP8 = mybir.dt.float8e4
E4 = ml_dtypes.float8_e4m3
F16_NP = np.float16
Alu = mybir.AluOpType
ActF = mybir.ActivationFunctionType
DR = mybir.MatmulPerfMode.DoubleRow

B, C, H1, H2, HO = 32768, 768, 512, 256, 256
DH, LAT, K = 256, 128, 8
N_CORES = 8
TILE = 512
MAIN_TILES = 8
MAIN = MAIN_TILES * TILE

nC, nH1, nH2, nDH = C // 128, H1 // 128, H2 // 128, DH // 128
P1, P2, PD = nC // 2, nH1 // 2, nH2 // 2  # contraction pairs per layer

# bias_all columns (with-bias variant): per layer, plain + s-premultiplied
OB1, OB1S = 0, 4            # L1: 4 cols each
OB2, OB2S = 8, 10           # L2: 2 cols
OAD1, OAD1S = 12, 14        # d1 set a
OAD2, OAD2S = 16, 18        # d2 set a
OAD3 = 20                   # d3 set a (1 col, identity)
OBD1, OBD1S = 21, 23
OBD2, OBD2S = 25, 27
OBD3 = 29
NBIAS = 30


def _build_nc(tail_w=32, with_bias=False, s2=0.0625, U=2.0**-13,
              n_warm=68, fills=(40, 30, 16, 110, 20, 20)):
    cap = MAIN + tail_w
    nc = bacc.Bacc(name="catnet_fp8")

    def din(name, shape):
        return nc.dram_tensor(name, shape, FP8, kind="ExternalInput")

    fthi = din("fthi", (C, MAIN))
    ftlo = din("ftlo", (C, MAIN))
    # weights arrive pre-rearranged to the SBUF layout: one contiguous DMA
    we1h = din("we1h", (128, nC, H1))
    we1l = din("we1l", (128, nC, H1))
    we2h = din("we2h", (128, nH1, H2))
    we2l = din("we2l", (128, nH1, H2))
    wd1ah = din("wd1ah", (128, nH2, DH))
    wd1al = din("wd1al", (128, nH2, DH))
    wd2ah = din("wd2ah", (128, nDH, DH))
    wd2al = din("wd2al", (128, nDH, DH))
    wd3ah = din("wd3ah", (128, nDH, LAT))
    wd3al = din("wd3al", (128, nDH, LAT))
    if tail_w:
        ftthi = din("ftthi", (C, tail_w))
        fttlo = din("fttlo", (C, tail_w))
        wd1bh = din("wd1bh", (128, nH2, DH))
        wd1bl = din("wd1bl", (128, nH2, DH))
        wd2bh = din("wd2bh", (128, nDH, DH))
        wd2bl = din("wd2bl", (128, nDH, DH))
        wd3bh = din("wd3bh", (128, nDH, LAT))
        wd3bl = din("wd3bl", (128, nDH, LAT))
    if with_bias:
        bias_all = nc.dram_tensor("bias_all", (128, NBIAS), F32,
                                  kind="ExternalInput")
    out = nc.dram_tensor("out", (LAT, cap), FP16, kind="ExternalOutput")


    with tile.TileContext(nc) as tc:
        with (
            tc.tile_pool(name="wp", bufs=1) as wp,
            tc.tile_pool(name="fp", bufs=2) as fp,
            tc.tile_pool(name="ap", bufs=4) as ap,
            tc.tile_pool(name="dp", bufs=2) as dp,
            tc.tile_pool(name="op", bufs=2) as op,
            tc.tile_pool(name="ps_enc", bufs=2, space="PSUM") as ps_enc,
            tc.tile_pool(name="ps_dec", bufs=2, space="PSUM") as ps_dec,
        ):
            # ---- PE warm-up: dummy matmuls cover the p-state ramp while
            # the first DMAs stream in. psum from the dec pool (its first
            # real tenant only appears at step 2).
            wz = wp.tile([128, 128], BF, tag="warmzero")
            nc.gpsimd.memset(wz[:], 0.0)
            wps = ps_dec.tile([128, 2, TILE], F32, tag="pd", name="pwwarm")
            for i in range(n_warm):
                nc.tensor.matmul(wps[:, 0, :64], wz[:, :128], wz[:, :64],
                                 start=(i == 0), stop=(i == n_warm - 1))

            def fill(n):
                if n <= 0:
                    return
                wpf = ps_dec.tile([128, 2, TILE], F32, tag="pd", name="pwfill")
                for i in range(n):
                    nc.tensor.matmul(wpf[:, 0, :64], wz[:, :128], wz[:, :64],
                                     start=(i == 0), stop=(i == n - 1))

            def prefill(ps, n):
                # dependency-free PE padding into the upcoming group's own
                # psum tile; discarded by the real group's start=True
                for i in range(n):
                    nc.tensor.matmul(ps[:, 0, :64], wz[:, :128], wz[:, :64],
                                     start=(i == 0), stop=(i == n - 1))

            # ---- startup DMAs (all sync queue). we1 hi/lo and ft0 hi/lo
            # interleave pair-by-pair so L1 tile 0 streams behind them.
            we1h_t = wp.tile([128, nC, H1], FP8, tag="we1h")
            we1l_t = wp.tile([128, nC, H1], FP8, tag="we1l")
            ft0h = fp.tile([128, nC, TILE], FP8, tag="fthi")
            ft0l = fp.tile([128, nC, TILE], FP8, tag="ftlo")
            fthi_r, ftlo_r = (t.rearrange("(c p) b -> p c b", p=128)
                              for t in (fthi, ftlo))
            nc.sync.dma_start(we1h_t[:], we1h[:])
            nc.sync.dma_start(we1l_t[:], we1l[:])
            for p in range(P1):
                sl = slice(2 * p, 2 * p + 2)
                nc.sync.dma_start(ft0h[:, sl, :], fthi_r[:, sl, 0:TILE])
                nc.sync.dma_start(ft0l[:, sl, :], ftlo_r[:, sl, 0:TILE])
            if with_bias:
                bias_t = wp.tile([128, NBIAS], F32, tag="bias")
                nc.sync.dma_start(bias_t[:], bias_all[:])
            we2h_t = wp.tile([128, nH1, H2], FP8, tag="we2h")
            we2l_t = wp.tile([128, nH1, H2], FP8, tag="we2l")
            wd1ah_t = wp.tile([128, nH2, DH], FP8, tag="wd1ah")
            wd1al_t = wp.tile([128, nH2, DH], FP8, tag="wd1al")
            wd2ah_t = wp.tile([128, nDH, DH], FP8, tag="wd2ah")
            wd2al_t = wp.tile([128, nDH, DH], FP8, tag="wd2al")
            wd3ah_t = wp.tile([128, nDH, LAT], FP8, tag="wd3ah")
            wd3al_t = wp.tile([128, nDH, LAT], FP8, tag="wd3al")
            if tail_w:
                wd1bh_t = wp.tile([128, nH2, DH], FP8, tag="wd1bh")
                wd1bl_t = wp.tile([128, nH2, DH], FP8, tag="wd1bl")
                wd2bh_t = wp.tile([128, nDH, DH], FP8, tag="wd2bh")
                wd2bl_t = wp.tile([128, nDH, DH], FP8, tag="wd2bl")
                wd3bh_t = wp.tile([128, nDH, LAT], FP8, tag="wd3bh")
                wd3bl_t = wp.tile([128, nDH, LAT], FP8, tag="wd3bl")

            # tile table: (col offset, width, weight set)
            tiles = [(t * TILE, TILE, "a") for t in range(MAIN_TILES)]
            if tail_w:
                tiles.append((MAIN, tail_w, "b"))
            nt = len(tiles)

            wsets = {"a": (wd1ah_t, wd1al_t, wd2ah_t, wd2al_t, wd3ah_t,
                           wd3al_t, OAD1, OAD2, OAD3)}
            if tail_w:
                wsets["b"] = (wd1bh_t, wd1bl_t, wd2bh_t, wd2bl_t, wd3bh_t,
                              wd3bl_t, OBD1, OBD2, OBD3)

            def dr_win(ps, w1_t, w2_t, xh, xl, pairs, mh, cs, wcol=None):
                if wcol is None:
                    wcol = lambda j: bass.ts(j, 128)
                cnt = [0] * mh
                tot = 3 * pairs
                def mm(j, w_t, x_t, p):
                    sl = slice(2 * p, 2 * p + 2)
                    nc.tensor.matmul(
                        ps[:, j, cs], w_t[:, sl, wcol(j)], x_t[:, sl, cs],
                        start=(cnt[j] == 0), stop=(cnt[j] == tot - 1),
                        perf_mode=DR)
                    cnt[j] += 1
                for p in range(pairs):
                    for j in range(mh):
                        mm(j, w1_t, xh, p)
                        mm(j, w2_t, xh, p)
                for p in range(pairs):
                    for j in range(mh):
                        mm(j, w1_t, xl, p)

            def dr_layer(ps, w1_t, w2_t, xh, xl, pairs, mh, tn, wcol=None):
                """Emit the 3-slot DR matmul set into fused psum `ps`
                ([128, mh, tn] halves). w*_t[:, pair-slice, 128-col-slice],
                xh/xl[:, pair-slice, :tn]. Slot3 (x_lo) emitted last."""
                if wcol is None:
                    wcol = lambda j: bass.ts(j, 128)
                cnt = [0] * mh
                tot = 3 * pairs
                def mm(j, w_t, x_t, p):
                    sl = slice(2 * p, 2 * p + 2)
                    nc.tensor.matmul(
                        ps[:, j, :tn], w_t[:, sl, wcol(j)], x_t[:, sl, :tn],
                        start=(cnt[j] == 0), stop=(cnt[j] == tot - 1),
                        perf_mode=DR)
                    cnt[j] += 1
                for p in range(pairs):
                    for j in range(mh):
                        mm(j, w1_t, xh, p)
                        mm(j, w2_t, xh, p)
                for p in range(pairs):
                    for j in range(mh):
                        mm(j, w1_t, xl, p)

            def act_AD(ps, hh, hl, cs, bcol, drain=False):
                sl_ = (slice(None), slice(None), cs)
                if with_bias:
                    nc.scalar.activation(hh[sl_], ps[sl_], ActF.Relu,
                                         bias=bias_t[:, bcol:bcol + 1])
                    h16 = ap.tile([128, 2, TILE], FP16, tag="h16bias")
                    nc.vector.tensor_scalar(h16[sl_], ps[sl_],
                                            bias_t[:, bcol:bcol + 1], 0.0,
                                            Alu.add, Alu.max)
                    eng = nc.vector if drain else nc.gpsimd
                    eng.tensor_tensor(hl[sl_], h16[sl_], hh[sl_],
                                      Alu.subtract)
                else:
                    nc.scalar.activation(hh[sl_], ps[sl_], ActF.Relu)
                    nc.vector.scalar_tensor_tensor(
                        hl[sl_], ps[sl_], 0.0, hh[sl_],
                        Alu.max, Alu.subtract)

            def act_BC(ps, hh, hl, h16, cs, s, bcol, bscol, drain=False):
                # ACT hh=e4m3(relu(p)*s) straight from psum; DVE
                # h16s=fp16(relu(p)*s); hl=e4m3(h16s-hh) on GP (tensor-
                # tensor, the only Pool-legal ALU form) or DVE in the drain
                sl_ = (slice(None), slice(None), cs)
                if with_bias:
                    nc.scalar.activation(hh[sl_], ps[sl_], ActF.Relu,
                                         scale=s,
                                         bias=bias_t[:, bscol:bscol + 1])
                    nc.scalar.activation(h16[sl_], ps[sl_], ActF.Relu,
                                         scale=s,
                                         bias=bias_t[:, bscol:bscol + 1])
                else:
                    nc.scalar.activation(hh[sl_], ps[sl_], ActF.Relu,
                                         scale=s)
                    nc.vector.tensor_scalar(h16[sl_], ps[sl_],
                                            s, 0.0, Alu.mult, Alu.max)
                eng = nc.vector if drain else nc.gpsimd
                eng.tensor_tensor(hl[sl_], h16[sl_], hh[sl_], Alu.subtract)

            def windows(tn, nsplit):
                w = tn // nsplit
                return [slice(i * w, (i + 1) * w) for i in range(nsplit)]

            # per-tile state
            a1 = [None] * nt   # [(hh,hl) x 2 fused pairs]
            a2 = [None] * nt   # (hh, hl)
            d1a = [None] * nt
            d2a = [None] * nt

            def ftile(t):
                if t == 0:
                    return ft0h, ft0l
                off, tn, _ = tiles[t]
                if tn == TILE:
                    fh = fp.tile([128, nC, TILE], FP8, tag="fthi")
                    fl = fp.tile([128, nC, TILE], FP8, tag="ftlo")
                    nc.sync.dma_start(fh[:], fthi_r[:, :, off:off + tn])
                    nc.sync.dma_start(fl[:], ftlo_r[:, :, off:off + tn])
                else:
                    fh = fp.tile([128, nC, tail_w], FP8, tag="ftthi", bufs=1)
                    fl = fp.tile([128, nC, tail_w], FP8, tag="fttlo", bufs=1)
                    nc.sync.dma_start(
                        fh[:], ftthi.rearrange("(c p) b -> p c b", p=128))
                    nc.sync.dma_start(
                        fl[:], fttlo.rearrange("(c p) b -> p c b", p=128))
                return fh, fl

            ft_cur = [None] * nt

            def emit_L1(t):
                off, tn, _ = tiles[t]
                fh, fl = ft_cur[t]
                acts = []
                for F in range(2):
                    ps = ps_enc.tile([128, 2, TILE], F32, tag="pe",
                                     name=f"psA{F}")
                    dr_layer(ps, we1h_t, we1l_t, fh, fl, P1, 2, tn,
                             wcol=lambda j, F=F: bass.ts(2 * F + j, 128))
                    hh = ap.tile([128, 2, TILE], FP8, tag="a1hh")
                    hl = ap.tile([128, 2, TILE], FP8, tag="a1hl")
                    act_AD(ps, hh, hl, slice(0, tn), OB1 + 2 * F,
                           drain=(t >= nt - 2))
                    acts.append((hh, hl))
                a1[t] = acts

            l2_state = [None]

            def emit_L2_p1(t):
                off, tn, _ = tiles[t]
                ps = ps_enc.tile([128, 2, TILE], F32, tag="pe", name="psB")
                cnt = [0] * 2
                def mm(j, w_t, x_t, p):
                    nc.tensor.matmul(
                        ps[:, j, :tn],
                        w_t[:, 2 * p:2 * p + 2, bass.ts(j, 128)],
                        x_t[:, :, :tn],
                        start=(cnt[j] == 0), stop=False, perf_mode=DR)
                    cnt[j] += 1
                for p in range(P2):
                    for j in range(2):
                        mm(j, we2h_t, a1[t][p][0], p)
                        mm(j, we2l_t, a1[t][p][0], p)
                l2_state[0] = ps

            def emit_L2_p2(t):
                off, tn, _ = tiles[t]
                ps = l2_state[0]
                cnt = [0] * 2
                def mm(j, w_t, x_t, p):
                    nc.tensor.matmul(
                        ps[:, j, :tn],
                        w_t[:, 2 * p:2 * p + 2, bass.ts(j, 128)],
                        x_t[:, :, :tn],
                        start=False, stop=(cnt[j] == P2 - 1), perf_mode=DR)
                    cnt[j] += 1
                for p in range(P2):
                    for j in range(2):
                        mm(j, we2h_t, a1[t][p][1], p)
                hh = ap.tile([128, 2, TILE], FP8, tag="a2hh")
                hl = ap.tile([128, 2, TILE], FP8, tag="a2hl")
                h16 = ap.tile([128, 2, TILE], FP16, tag="h16b")
                act_BC(ps, hh, hl, h16, slice(0, tn), s2, OB2, OB2S,
                       drain=(t >= nt - 2))
                a2[t] = (hh, hl)

            def emit_d1(t, pre=0, nsplit=1):
                off, tn, ws = tiles[t]
                w1_t, w2_t = wsets[ws][0], wsets[ws][1]
                od1 = wsets[ws][6]
                ps = ps_dec.tile([128, 2, TILE], F32, tag="pd", name="psD")
                prefill(ps, pre)
                hh = dp.tile([128, 2, TILE], FP8, tag="d1hh")
                hl = dp.tile([128, 2, TILE], FP8, tag="d1hl")
                for cs in windows(tn, nsplit):
                    dr_win(ps, w1_t, w2_t, a2[t][0], a2[t][1], PD, 2, cs)
                    act_AD(ps, hh, hl, cs, od1, drain=(t >= nt - 2))
                d1a[t] = (hh, hl)

            def emit_d2(t, nsplit=1):
                off, tn, ws = tiles[t]
                w1_t, w2_t = wsets[ws][2], wsets[ws][3]
                od2 = wsets[ws][7]
                ps = ps_dec.tile([128, 2, TILE], F32, tag="pd", name="psC")
                hh = dp.tile([128, 2, TILE], FP8, tag="d2hh")
                hl = dp.tile([128, 2, TILE], FP8, tag="d2hl")
                for cs in windows(tn, nsplit):
                    dr_win(ps, w1_t, w2_t, d1a[t][0], d1a[t][1], PD, 2, cs)
                    act_AD(ps, hh, hl, cs, od2, drain=(t >= nt - 2))
                d2a[t] = (hh, hl)

            osbw = [None]

            def emit_d3(t, nsplit=1):
                off, tn, ws = tiles[t]
                w1_t, w2_t = wsets[ws][4], wsets[ws][5]
                od3 = wsets[ws][8]
                ps = ps_dec.tile([128, TILE], F32, tag="pd", name="psE")
                if tail_w and t == nt - 2:
                    osb = op.tile([128, TILE + tail_w], FP16, tag="osbw",
                                  bufs=1)
                    osbw[0] = osb
                elif tail_w and t == nt - 1:
                    osb = osbw[0]
                else:
                    osb = op.tile([128, TILE], FP16, tag="osb")
                for cs in windows(tn, nsplit):
                    cnt = 0
                    for x_t, w_t in ((d2a[t][0], w1_t), (d2a[t][0], w2_t),
                                     (d2a[t][1], w1_t)):
                        nc.tensor.matmul(ps[:, cs], w_t[:, :, :],
                                         x_t[:, :, cs],
                                         start=(cnt == 0), stop=(cnt == 2),
                                         perf_mode=DR)
                        cnt += 1
                    if tail_w and t == nt - 1:
                        # tail copy-out on the (idle) DVE
                        osl = slice(TILE + cs.start, TILE + cs.stop)
                        if with_bias:
                            nc.vector.tensor_scalar(osb[:, osl], ps[:, cs],
                                                    U,
                                                    bias_t[:, od3:od3 + 1],
                                                    Alu.mult, Alu.add)
                        else:
                            nc.vector.tensor_scalar(osb[:, osl], ps[:, cs],
                                                    U, None, Alu.mult)
                    elif with_bias:
                        nc.scalar.activation(osb[:, cs], ps[:, cs],
                                             ActF.Identity,
                                             bias=bias_t[:, od3:od3 + 1],
                                             scale=U)
                    else:
                        nc.scalar.activation(osb[:, cs], ps[:, cs],
                                             ActF.Identity, scale=U)
                if tail_w and t == nt - 1:
                    off7 = tiles[nt - 2][0]
                    nc.sync.dma_start(out[:, off7:off7 + TILE + tn],
                                      osb[:, :TILE + tn])
                elif not (tail_w and t == nt - 2):
                    nc.sync.dma_start(out[:, off:off + tn], osb[:, :tn])

            # ---- software pipeline: per step t
            # [L1(t)][d3(t-3)][d2(t-2)][L2(t)][d1(t-1)], with the last big
            # tile's d1 pulled into its own step (lag-0, fill-covered wait)
            # so the drain ladder starts two steps early.
            ft_cur[0] = (ft0h, ft0l)
            d1_done = set()
            for t in range(nt):
                emit_L1(t)
                if t == 0:
                    nc.sync.dma_start(we2h_t[:], we2h[:])
                    nc.sync.dma_start(we2l_t[:], we2l[:])
                    nc.sync.dma_start(wd1ah_t[:], wd1ah[:])
                    nc.sync.dma_start(wd1al_t[:], wd1al[:])
                if t + 1 < nt:
                    ft_cur[t + 1] = ftile(t + 1)
                if t == 0:
                    nc.sync.dma_start(wd2ah_t[:], wd2ah[:])
                    nc.sync.dma_start(wd2al_t[:], wd2al[:])
                    nc.sync.dma_start(wd3ah_t[:], wd3ah[:])
                    nc.sync.dma_start(wd3al_t[:], wd3al[:])
                elif t == 1 and tail_w:
                    nc.sync.dma_start(wd1bh_t[:], wd1bh[:])
                    nc.sync.dma_start(wd1bl_t[:], wd1bl[:])
                    nc.sync.dma_start(wd2bh_t[:], wd2bh[:])
                    nc.sync.dma_start(wd2bl_t[:], wd2bl[:])
                    nc.sync.dma_start(wd3bh_t[:], wd3bh[:])
                    nc.sync.dma_start(wd3bl_t[:], wd3bl[:])
                if 0 <= t - 3:
                    emit_d3(t - 3)
                if 0 <= t - 2:
                    emit_d2(t - 2)
                if t == 0:
                    fill(fills[0])
                elif t == 1:
                    fill(fills[1])
                emit_L2_p1(t)
                if 0 <= t - 1 and t - 1 not in d1_done:
                    if t == 1:
                        fill(fills[2])
                    emit_d1(t - 1)
                    d1_done.add(t - 1)
                emit_L2_p2(t)
                if t == nt - 2 and t not in d1_done:
                    emit_d1(t, pre=fills[3])
                    d1_done.add(t)
            # ---- drain
            emit_d2(nt - 2)
            if nt - 1 not in d1_done:
                emit_d1(nt - 1)
            emit_d3(nt - 3)
            fill(fills[4])
            emit_d2(nt - 1)
            emit_d3(nt - 2)
            fill(fills[5])
            emit_d3(nt - 1)

    nc.finalize()
    return nc


def _plan_tails(counts):
    """Assign overflow rows (beyond MAIN per category) to per-core tail
    slots: one category per core tail, tail_w rows max per core."""
    ov = {k: int(c) - MAIN for k, c in enumerate(counts) if c > MAIN}
    if not ov:
        return 0, [None] * N_CORES
    for tail_w in (32, 64, 128, 256, 384, 512):
        if sum(-(-v // tail_w) for v in ov.values()) <= N_CORES:
            break
    else:
        return None, None
    assign = []  # (cat, n_rows) per used core
    for k, v in sorted(ov.items()):
        while v > 0:
            take = min(v, tail_w)
            assign.append((k, take))
            v -= take
    assign += [None] * (N_CORES - len(assign))
    return tail_w, assign


def _pow2(v):
    return float(2.0 ** np.round(np.log2(max(v, 1e-30))))


def _split8(x):
    hi = np.asarray(x, np.float32).astype(E4)
    lo = (np.asarray(x, np.float32) - hi.astype(np.float32)).astype(E4)
    return hi, lo


def _wpack(w):
    """(C_, M) -> [128, C_/128, M] SBUF layout for single-DMA loads."""
    w = np.asarray(w)
    n = w.shape[0] // 128
    return np.ascontiguousarray(w.reshape(n, 128, -1).transpose(1, 0, 2))


def _chunkcols(b):
    return np.asarray(b, np.float32).reshape(-1).reshape(-1, 128).T


def _scales(features, We1f, We2f, We3f, Wd1f, Wd2f, Wd3f):
    """Shared pow2 scale chain (identical across cores; the graph bakes
    s2/s4/U as immediates and is cached per scale-tuple)."""
    sig_f = float(np.asarray(features, np.float32)[::37].std())
    c1 = _pow2(0.5 / sig_f)
    z1 = _pow2(0.35 / float(We1f.std()))
    z2 = _pow2(0.35 / float(We2f.std()))
    Wfs = [We3f @ Wd1f[k] for k in range(K)]
    z3 = _pow2(0.35 / max(float(w.std()) for w in Wfs))
    z4 = _pow2(0.35 / max(float(Wd2f[k].std()) for k in range(K)))
    z5 = _pow2(0.35 / max(float(Wd3f[k].std()) for k in range(K)))
    S1 = c1 * sig_f * z1 * np.linalg.norm(We1f) / np.sqrt(H1)
    r2 = S1 / np.sqrt(2)
    S2 = r2 * z2 * np.linalg.norm(We2f) / np.sqrt(H2)
    s2 = _pow2(1.2 / (S2 / np.sqrt(2)))
    U = 1.0 / (c1 * z1 * z2 * s2 * z3 * z4 * z5)
    return c1, z1, z2, z3, z4, z5, s2, U


def _pack_inputs(features, We1, be1, We2, be2, We3, be3,
                 Wd1, bd1, Wd2, bd2, Wd3, bd3, cat_idx,
                 tail_w, tails, with_bias):
    features = np.asarray(features, np.float32)
    cat = np.asarray(cat_idx).astype(np.int64)
    order = np.argsort(cat, kind="stable")
    counts = np.bincount(cat, minlength=N_CORES)
    starts = np.zeros(N_CORES + 1, np.int64)
    np.cumsum(counts, out=starts[1:])
    cat_rows = [order[starts[k]:starts[k + 1]] for k in range(N_CORES)]

    We1f = np.asarray(We1, np.float32)
    We2f = np.asarray(We2, np.float32)
    We3f = np.asarray(We3, np.float32)
    be3f = np.asarray(be3, np.float32)
    Wd1f = np.asarray(Wd1, np.float32)
    bd1f = np.asarray(bd1, np.float32)
    Wd2f = np.asarray(Wd2, np.float32)
    bd2f = np.asarray(bd2, np.float32)
    Wd3f = np.asarray(Wd3, np.float32)
    bd3f = np.asarray(bd3, np.float32)

    c1, z1, z2, z3, z4, z5, s2, U = _scales(
        features, We1f, We2f, We3f, Wd1f, Wd2f, Wd3f)
    Wfs = [We3f @ Wd1f[k] for k in range(K)]

    # cumulative psum pre-scales per layer (for bias columns)
    Pm1 = c1 * z1
    Pm2 = Pm1 * z2
    Pm3 = Pm2 * s2 * z3
    Pm4 = Pm3 * z4
    Pm5 = Pm4 * z5

    we1h_v, we1l_v = (_wpack(w) for w in _split8(We1f * z1))
    we2h_v, we2l_v = (_wpack(w) for w in _split8(We2f * z2))

    def dec_weights(k):
        wf = Wfs[k]
        d1h, d1l = _split8(wf * z3)
        d2h, d2l = _split8(Wd2f[k] * z4)
        d3h, d3l = _split8(Wd3f[k] * z5)
        return tuple(_wpack(w) for w in (d1h, d1l, d2h, d2l, d3h, d3l))

    def dec_bias(k):
        b1 = np.asarray(Wd1f[k]).T @ be3f + bd1f[k]
        return (_chunkcols(b1 * Pm3), _chunkcols(bd2f[k] * Pm4),
                _chunkcols(bd3f[k] * Pm5))

    used = {k: MAIN for k in range(N_CORES)}
    maps, row_maps = [], []
    for j in range(N_CORES):
        main_rows = cat_rows[j][:MAIN]
        f = np.zeros((MAIN, C), np.float32)
        f[:len(main_rows)] = features[main_rows]
        fT = np.ascontiguousarray(f.T) * c1
        fh, fl = _split8(fT)
        tail_rows = np.empty((0,), np.int64)
        tcat = j
        if tail_w and tails[j] is not None:
            tcat, n = tails[j]
            tail_rows = cat_rows[tcat][used[tcat]:used[tcat] + n]
            used[tcat] += n
        da = dec_weights(j)
        m = {
            "fthi": fh, "ftlo": fl,
            "we1h": we1h_v, "we1l": we1l_v,
            "we2h": we2h_v, "we2l": we2l_v,
            "wd1ah": da[0], "wd1al": da[1],
            "wd2ah": da[2], "wd2al": da[3],
            "wd3ah": da[4], "wd3al": da[5],
        }
        if tail_w:
            ft = np.zeros((tail_w, C), np.float32)
            ft[:len(tail_rows)] = features[tail_rows]
            ftT = np.ascontiguousarray(ft.T) * c1
            th, tl = _split8(ftT)
            db = dec_weights(tcat)
            m["ftthi"], m["fttlo"] = th, tl
            m["wd1bh"], m["wd1bl"] = db[0], db[1]
            m["wd2bh"], m["wd2bl"] = db[2], db[3]
            m["wd3bh"], m["wd3bl"] = db[4], db[5]
        if with_bias:
            bias_all = np.zeros((128, NBIAS), np.float32)
            bias_all[:, OB1:OB1 + 4] = _chunkcols(np.asarray(be1) * Pm1)
            bias_all[:, OB1S:OB1S + 4] = bias_all[:, OB1:OB1 + 4]
            bias_all[:, OB2:OB2 + 2] = _chunkcols(np.asarray(be2) * Pm2)
            bias_all[:, OB2S:OB2S + 2] = bias_all[:, OB2:OB2 + 2] * s2
            ba = dec_bias(j)
            bias_all[:, OAD1:OAD1 + 2] = ba[0]
            bias_all[:, OAD1S:OAD1S + 2] = ba[0]
            bias_all[:, OAD2:OAD2 + 2] = ba[1]
            bias_all[:, OAD3:OAD3 + 1] = ba[2] * U
            if tail_w:
                bb = dec_bias(tcat)
                bias_all[:, OBD1:OBD1 + 2] = bb[0]
                bias_all[:, OBD1S:OBD1S + 2] = bb[0]
                bias_all[:, OBD2:OBD2 + 2] = bb[1]
                bias_all[:, OBD3:OBD3 + 1] = bb[2] * U
            m["bias_all"] = bias_all
        maps.append(m)
        row_maps.append((main_rows, tail_rows))
    return maps, row_maps


_NC_CACHE = {}
_LAST_KEY = None


def _get_nc(key=None):
    global _LAST_KEY
    if key is None:
        key = _LAST_KEY if _LAST_KEY is not None else (
            32, False, 0.0625, 2.0 ** -13)
    if key not in _NC_CACHE:
        _NC_CACHE[key] = _build_nc(*key)
    _LAST_KEY = key
    return _NC_CACHE[key]


def kernel(**inputs) -> np.ndarray:
    cat = np.asarray(inputs["cat_idx"]).astype(np.int64)
    counts = np.bincount(cat, minlength=K)
    tail_w, tails = _plan_tails(counts)
    assert tail_w is not None, "category distribution too skewed for tails"
    with_bias = any(
        np.any(np.asarray(inputs[k], np.float32))
        for k in ("be1", "be2", "be3", "bd1", "bd2", "bd3"))
    sc = _scales(np.asarray(inputs["features"], np.float32),
                 np.asarray(inputs["We1"], np.float32),
                 np.asarray(inputs["We2"], np.float32),
                 np.asarray(inputs["We3"], np.float32),
                 np.asarray(inputs["Wd1"], np.float32),
                 np.asarray(inputs["Wd2"], np.float32),
                 np.asarray(inputs["Wd3"], np.float32))
    nc = _get_nc((tail_w, with_bias, sc[6], sc[7]))
    maps, row_maps = _pack_inputs(**inputs, tail_w=tail_w, tails=tails,
                                  with_bias=with_bias)
    res = bass_utils.run_bass_kernel_spmd(nc, maps, core_ids=list(range(N_CORES)))
    latent = np.zeros((B, LAT), np.float32)
    for j, r in enumerate(res.results):
        main_rows, tail_rows = row_maps[j]
        o = np.asarray(r["out"]).astype(np.float32)
        latent[main_rows] = o[:, :len(main_rows)].T
        if len(tail_rows):
            latent[tail_rows] = o[:, MAIN:MAIN + len(tail_rows)].T
    return latent


# revision 54
# speedup vs baseline: 1.0149x; 1.0149x over previous
"""Trainium2 Bass kernel for nn_CategorySpecificInitNet (moe_routing).

kernel(**inputs) takes the FULL unsharded inputs (keys as in
reference.setup_inputs()) and returns the FULL [B, 128] float32 output.

Strategy — expert-parallel + fp8 DoubleRow compensated matmuls:
  - rows are dispatched to cores by category: core k's MAIN block is 4096
    rows (8 tiles x 512) of category k, so the main block runs ONE decoder
    with no routing/masking; categories with more than 4096 rows spill
    into small per-core TAIL tiles (decoder weight set "B");
  - the encoder's third linear layer is constant-folded into each
    decoder's first layer on the host (W_f = We3 @ Wd1_k, exact algebra);
  - every GEMM runs as fp8e4 (e4m3) matmuls in DoubleRow perf mode
    (2 contraction rows per PE cell), with a 3-slot error-compensated
    split: w = w_hi + w_lo, x = x_hi + x_lo (each e4m3), computing
    w_hi*x_hi + w_lo*x_hi + w_hi*x_lo. Per 256 contraction rows this is
    3 DR matmuls instead of 2 plain bf16 matmuls = 0.75x PE cycles at
    ~bf16-grade accuracy (the dropped w_lo*x_lo term is ~2^-8 relative);
  - power-of-2 scaling keeps every e4m3 quantization point in the
    format's sweet spot: weights scaled to sigma~0.35, features to ~0.5,
    and the psum scale grows geometrically layer to layer (all quantize
    points stay under e4m3's +-240 ceiling), reset by mid-net rescales at
    the L2 and d2 outputs; the final fp32 psum is unwound by a single
    pow2 scale in the copy-out;
  - activation hi/lo streams are produced straight from PSUM:
      A/D blocks (L1, d1 outs):  ACT hh=e4m3(relu(p)),
                                 DVE hl=e4m3(relu(p)-hh)  [scalar_tensor_tensor]
      B/C blocks (L2, d2 outs):  ACT hh=e4m3(relu(p)*s), DVE h16=fp16(relu(p)*s),
                                 GPSIMD hl=e4m3(h16-hh)   [rescale rides free]
    one op per engine per block, fitting under the PE time per tile;
  - psum tiles are fused chunk-pairs [128, 2, 512] (2 banks) so one
    ACT/DVE op covers a DoubleRow pair and the activation tiles are
    directly the [128, 2, N] DR moving operand;
  - software pipeline per step t: [L1(t)][d2(t-2)][d3(t-3)][L2(t)][d1(t-1)]
    hides the ~1.4/2.7us hh/hl latency chains behind matmuls of other
    tiles; 2+2 psum pair-pool slots (8 banks) exactly cover the rotation;
  - a warm-up chain of dummy matmuls covers the tensor engine p-state
    ramp while the first weight/feature DMAs stream in (all on the sync
    queue, interleaved pair-by-pair so L1 tile 0 streams behind them).

Measured end-to-end error vs the fp32 reference: ~4.6e-3 max-rel (gate 2e-2).
"""
import sys

for _p in ("/opt/trn_rl_repo",):
    if _p not in sys.path:
        sys.path.append(_p)

import numpy as np
import ml_dtypes

import concourse.bass as bass
import concourse.bacc as bacc
import concourse.mybir as mybir
import concourse.tile as tile
from concourse import bass_utils

F32 = mybir.dt.float32
BF = mybir.dt.bfloat16
FP16 = mybir.dt.float16
FP8 = mybir.dt.float8e4
E4 = ml_dtypes.float8_e4m3
F16_NP = np.float16
Alu = mybir.AluOpType
ActF = mybir.ActivationFunctionType
DR = mybir.MatmulPerfMode.DoubleRow

B, C, H1, H2, HO = 32768, 768, 512, 256, 256
DH, LAT, K = 256, 128, 8
N_CORES = 8
TILE = 512
MAIN_TILES = 8
MAIN = MAIN_TILES * TILE

nC, nH1, nH2, nDH = C // 128, H1 // 128, H2 // 128, DH // 128
P1, P2, PD = nC // 2, nH1 // 2, nH2 // 2  # contraction pairs per layer

# bias_all columns (with-bias variant): per layer, plain + s-premultiplied
OB1, OB1S = 0, 4            # L1: 4 cols each
OB2, OB2S = 8, 10           # L2: 2 cols
OAD1, OAD1S = 12, 14        # d1 set a
OAD2, OAD2S = 16, 18        # d2 set a
OAD3 = 20                   # d3 set a (1 col, identity)
OBD1, OBD1S = 21, 23
OBD2, OBD2S = 25, 27
OBD3 = 29
NBIAS = 30


def _build_nc(tail_w=32, with_bias=False, s2=0.0625, U=2.0**-13,
              n_warm=45, fills=(40, 30, 16, 110, 20, 20)):
    cap = MAIN + tail_w
    nc = bacc.Bacc(name="catnet_fp8")

    def din(name, shape):
        return nc.dram_tensor(name, shape, FP8, kind="ExternalInput")

    fthi = din("fthi", (C, MAIN))
    ftlo = din("ftlo", (C, MAIN))
    # weights arrive pre-rearranged to the SBUF layout: one contiguous DMA
    we1h = din("we1h", (128, nC, H1))
    we1l = din("we1l", (128, nC, H1))
    we2h = din("we2h", (128, nH1, H2))
    we2l = din("we2l", (128, nH1, H2))
    wd1ah = din("wd1ah", (128, nH2, DH))
    wd1al = din("wd1al", (128, nH2, DH))
    wd2ah = din("wd2ah", (128, nDH, DH))
    wd2al = din("wd2al", (128, nDH, DH))
    wd3ah = din("wd3ah", (128, nDH, LAT))
    wd3al = din("wd3al", (128, nDH, LAT))
    if tail_w:
        ftthi = din("ftthi", (C, tail_w))
        fttlo = din("fttlo", (C, tail_w))
        wd1bh = din("wd1bh", (128, nH2, DH))
        wd1bl = din("wd1bl", (128, nH2, DH))
        wd2bh = din("wd2bh", (128, nDH, DH))
        wd2bl = din("wd2bl", (128, nDH, DH))
        wd3bh = din("wd3bh", (128, nDH, LAT))
        wd3bl = din("wd3bl", (128, nDH, LAT))
    if with_bias:
        bias_all = nc.dram_tensor("bias_all", (128, NBIAS), F32,
                                  kind="ExternalInput")
    out = nc.dram_tensor("out", (LAT, cap), FP16, kind="ExternalOutput")


    with tile.TileContext(nc) as tc:
        with (
            tc.tile_pool(name="wp", bufs=1) as wp,
            tc.tile_pool(name="fp", bufs=2) as fp,
            tc.tile_pool(name="ap", bufs=4) as ap,
            tc.tile_pool(name="dp", bufs=2) as dp,
            tc.tile_pool(name="op", bufs=2) as op,
            tc.tile_pool(name="ps_enc", bufs=2, space="PSUM") as ps_enc,
            tc.tile_pool(name="ps_dec", bufs=2, space="PSUM") as ps_dec,
        ):
            # ---- PE warm-up: dummy matmuls cover the p-state ramp while
            # the first DMAs stream in. psum from the dec pool (its first
            # real tenant only appears at step 2).
            wz = wp.tile([128, 128], BF, tag="warmzero")
            nc.gpsimd.memset(wz[:], 0.0)
            wps = ps_dec.tile([128, 2, TILE], F32, tag="pd", name="pwwarm")
            for i in range(n_warm):
                nc.tensor.matmul(wps[:, 0, :64], wz[:, :128], wz[:, :64],
                                 start=(i == 0), stop=(i == n_warm - 1))

            def fill(n):
                if n <= 0:
                    return
                wpf = ps_dec.tile([128, 2, TILE], F32, tag="pd", name="pwfill")
                for i in range(n):
                    nc.tensor.matmul(wpf[:, 0, :64], wz[:, :128], wz[:, :64],
                                     start=(i == 0), stop=(i == n - 1))

            def prefill(ps, n):
                # dependency-free PE padding into the upcoming group's own
                # psum tile; discarded by the real group's start=True
                for i in range(n):
                    nc.tensor.matmul(ps[:, 0, :64], wz[:, :128], wz[:, :64],
                                     start=(i == 0), stop=(i == n - 1))

            # ---- startup DMAs (all sync queue). we1 hi/lo and ft0 hi/lo
            # interleave pair-by-pair so L1 tile 0 streams behind them.
            we1h_t = wp.tile([128, nC, H1], FP8, tag="we1h")
            we1l_t = wp.tile([128, nC, H1], FP8, tag="we1l")
            ft0h = fp.tile([128, nC, TILE], FP8, tag="fthi")
            ft0l = fp.tile([128, nC, TILE], FP8, tag="ftlo")
            fthi_r, ftlo_r = (t.rearrange("(c p) b -> p c b", p=128)
                              for t in (fthi, ftlo))
            nc.sync.dma_start(we1h_t[:], we1h[:])
            nc.sync.dma_start(we1l_t[:], we1l[:])
            for p in range(P1):
                sl = slice(2 * p, 2 * p + 2)
                nc.sync.dma_start(ft0h[:, sl, :], fthi_r[:, sl, 0:TILE])
                nc.sync.dma_start(ft0l[:, sl, :], ftlo_r[:, sl, 0:TILE])
            if with_bias:
                bias_t = wp.tile([128, NBIAS], F32, tag="bias")
                nc.sync.dma_start(bias_t[:], bias_all[:])
            we2h_t = wp.tile([128, nH1, H2], FP8, tag="we2h")
            we2l_t = wp.tile([128, nH1, H2], FP8, tag="we2l")
            wd1ah_t = wp.tile([128, nH2, DH], FP8, tag="wd1ah")
            wd1al_t = wp.tile([128, nH2, DH], FP8, tag="wd1al")
            wd2ah_t = wp.tile([128, nDH, DH], FP8, tag="wd2ah")
            wd2al_t = wp.tile([128, nDH, DH], FP8, tag="wd2al")
            wd3ah_t = wp.tile([128, nDH, LAT], FP8, tag="wd3ah")
            wd3al_t = wp.tile([128, nDH, LAT], FP8, tag="wd3al")
            if tail_w:
                wd1bh_t = wp.tile([128, nH2, DH], FP8, tag="wd1bh")
                wd1bl_t = wp.tile([128, nH2, DH], FP8, tag="wd1bl")
                wd2bh_t = wp.tile([128, nDH, DH], FP8, tag="wd2bh")
                wd2bl_t = wp.tile([128, nDH, DH], FP8, tag="wd2bl")
                wd3bh_t = wp.tile([128, nDH, LAT], FP8, tag="wd3bh")
                wd3bl_t = wp.tile([128, nDH, LAT], FP8, tag="wd3bl")

            # tile table: (col offset, width, weight set)
            tiles = [(t * TILE, TILE, "a") for t in range(MAIN_TILES)]
            if tail_w:
                tiles.append((MAIN, tail_w, "b"))
            nt = len(tiles)

            wsets = {"a": (wd1ah_t, wd1al_t, wd2ah_t, wd2al_t, wd3ah_t,
                           wd3al_t, OAD1, OAD2, OAD3)}
            if tail_w:
                wsets["b"] = (wd1bh_t, wd1bl_t, wd2bh_t, wd2bl_t, wd3bh_t,
                              wd3bl_t, OBD1, OBD2, OBD3)

            def dr_win(ps, w1_t, w2_t, xh, xl, pairs, mh, cs, wcol=None):
                if wcol is None:
                    wcol = lambda j: bass.ts(j, 128)
                cnt = [0] * mh
                tot = 3 * pairs
                def mm(j, w_t, x_t, p):
                    sl = slice(2 * p, 2 * p + 2)
                    nc.tensor.matmul(
                        ps[:, j, cs], w_t[:, sl, wcol(j)], x_t[:, sl, cs],
                        start=(cnt[j] == 0), stop=(cnt[j] == tot - 1),
                        perf_mode=DR)
                    cnt[j] += 1
                for p in range(pairs):
                    for j in range(mh):
                        mm(j, w1_t, xh, p)
                        mm(j, w2_t, xh, p)
                for p in range(pairs):
                    for j in range(mh):
                        mm(j, w1_t, xl, p)

            def dr_layer(ps, w1_t, w2_t, xh, xl, pairs, mh, tn, wcol=None):
                """Emit the 3-slot DR matmul set into fused psum `ps`
                ([128, mh, tn] halves). w*_t[:, pair-slice, 128-col-slice],
                xh/xl[:, pair-slice, :tn]. Slot3 (x_lo) emitted last."""
                if wcol is None:
                    wcol = lambda j: bass.ts(j, 128)
                cnt = [0] * mh
                tot = 3 * pairs
                def mm(j, w_t, x_t, p):
                    sl = slice(2 * p, 2 * p + 2)
                    nc.tensor.matmul(
                        ps[:, j, :tn], w_t[:, sl, wcol(j)], x_t[:, sl, :tn],
                        start=(cnt[j] == 0), stop=(cnt[j] == tot - 1),
                        perf_mode=DR)
                    cnt[j] += 1
                for p in range(pairs):
                    for j in range(mh):
                        mm(j, w1_t, xh, p)
                        mm(j, w2_t, xh, p)
                for p in range(pairs):
                    for j in range(mh):
                        mm(j, w1_t, xl, p)

            def act_AD(ps, hh, hl, cs, bcol, drain=False):
                sl_ = (slice(None), slice(None), cs)
                if with_bias:
                    nc.scalar.activation(hh[sl_], ps[sl_], ActF.Relu,
                                         bias=bias_t[:, bcol:bcol + 1])
                    h16 = ap.tile([128, 2, TILE], FP16, tag="h16bias")
                    nc.vector.tensor_scalar(h16[sl_], ps[sl_],
                                            bias_t[:, bcol:bcol + 1], 0.0,
                                            Alu.add, Alu.max)
                    eng = nc.vector if drain else nc.gpsimd
                    eng.tensor_tensor(hl[sl_], h16[sl_], hh[sl_],
                                      Alu.subtract)
                else:
                    nc.scalar.activation(hh[sl_], ps[sl_], ActF.Relu)
                    nc.vector.scalar_tensor_tensor(
                        hl[sl_], ps[sl_], 0.0, hh[sl_],
                        Alu.max, Alu.subtract)

            def act_BC(ps, hh, hl, h16, cs, s, bcol, bscol, drain=False):
                # ACT hh=e4m3(relu(p)*s) straight from psum; DVE
                # h16s=fp16(relu(p)*s); hl=e4m3(h16s-hh) on GP (tensor-
                # tensor, the only Pool-legal ALU form) or DVE in the drain
                sl_ = (slice(None), slice(None), cs)
                if with_bias:
                    nc.scalar.activation(hh[sl_], ps[sl_], ActF.Relu,
                                         scale=s,
                                         bias=bias_t[:, bscol:bscol + 1])
                    nc.scalar.activation(h16[sl_], ps[sl_], ActF.Relu,
                                         scale=s,
                                         bias=bias_t[:, bscol:bscol + 1])
                else:
                    nc.scalar.activation(hh[sl_], ps[sl_], ActF.Relu,
                                         scale=s)
                    nc.vector.tensor_scalar(h16[sl_], ps[sl_],
                                            s, 0.0, Alu.mult, Alu.max)
                eng = nc.vector if drain else nc.gpsimd
                eng.tensor_tensor(hl[sl_], h16[sl_], hh[sl_], Alu.subtract)

            def windows(tn, nsplit):
                w = tn // nsplit
                return [slice(i * w, (i + 1) * w) for i in range(nsplit)]

            # per-tile state
            a1 = [None] * nt   # [(hh,hl) x 2 fused pairs]
            a2 = [None] * nt   # (hh, hl)
            d1a = [None] * nt
            d2a = [None] * nt

            def ftile(t):
                if t == 0:
                    return ft0h, ft0l
                off, tn, _ = tiles[t]
                if tn == TILE:
                    fh = fp.tile([128, nC, TILE], FP8, tag="fthi")
                    fl = fp.tile([128, nC, TILE], FP8, tag="ftlo")
                    nc.sync.dma_start(fh[:], fthi_r[:, :, off:off + tn])
                    nc.sync.dma_start(fl[:], ftlo_r[:, :, off:off + tn])
                else:
                    fh = fp.tile([128, nC, tail_w], FP8, tag="ftthi", bufs=1)
                    fl = fp.tile([128, nC, tail_w], FP8, tag="fttlo", bufs=1)
                    nc.sync.dma_start(
                        fh[:], ftthi.rearrange("(c p) b -> p c b", p=128))
                    nc.sync.dma_start(
                        fl[:], fttlo.rearrange("(c p) b -> p c b", p=128))
                return fh, fl

            ft_cur = [None] * nt

            def emit_L1(t):
                off, tn, _ = tiles[t]
                fh, fl = ft_cur[t]
                acts = []
                for F in range(2):
                    ps = ps_enc.tile([128, 2, TILE], F32, tag="pe",
                                     name=f"psA{F}")
                    dr_layer(ps, we1h_t, we1l_t, fh, fl, P1, 2, tn,
                             wcol=lambda j, F=F: bass.ts(2 * F + j, 128))
                    hh = ap.tile([128, 2, TILE], FP8, tag="a1hh")
                    hl = ap.tile([128, 2, TILE], FP8, tag="a1hl")
                    act_AD(ps, hh, hl, slice(0, tn), OB1 + 2 * F,
                           drain=(t >= nt - 2))
                    acts.append((hh, hl))
                a1[t] = acts

            l2_state = [None]

            def emit_L2_p1(t):
                off, tn, _ = tiles[t]
                ps = ps_enc.tile([128, 2, TILE], F32, tag="pe", name="psB")
                cnt = [0] * 2
                def mm(j, w_t, x_t, p):
                    nc.tensor.matmul(
                        ps[:, j, :tn],
                        w_t[:, 2 * p:2 * p + 2, bass.ts(j, 128)],
                        x_t[:, :, :tn],
                        start=(cnt[j] == 0), stop=False, perf_mode=DR)
                    cnt[j] += 1
                for p in range(P2):
                    for j in range(2):
                        mm(j, we2h_t, a1[t][p][0], p)
                        mm(j, we2l_t, a1[t][p][0], p)
                l2_state[0] = ps

            def emit_L2_p2(t):
                off, tn, _ = tiles[t]
                ps = l2_state[0]
                cnt = [0] * 2
                def mm(j, w_t, x_t, p):
                    nc.tensor.matmul(
                        ps[:, j, :tn],
                        w_t[:, 2 * p:2 * p + 2, bass.ts(j, 128)],
                        x_t[:, :, :tn],
                        start=False, stop=(cnt[j] == P2 - 1), perf_mode=DR)
                    cnt[j] += 1
                for p in range(P2):
                    for j in range(2):
                        mm(j, we2h_t, a1[t][p][1], p)
                hh = ap.tile([128, 2, TILE], FP8, tag="a2hh")
                hl = ap.tile([128, 2, TILE], FP8, tag="a2hl")
                h16 = ap.tile([128, 2, TILE], FP16, tag="h16b")
                act_BC(ps, hh, hl, h16, slice(0, tn), s2, OB2, OB2S,
                       drain=(t >= nt - 2))
                a2[t] = (hh, hl)

            def emit_d1(t, pre=0, nsplit=1):
                off, tn, ws = tiles[t]
                w1_t, w2_t = wsets[ws][0], wsets[ws][1]
                od1 = wsets[ws][6]
                ps = ps_dec.tile([128, 2, TILE], F32, tag="pd", name="psD")
                prefill(ps, pre)
                hh = dp.tile([128, 2, TILE], FP8, tag="d1hh")
                hl = dp.tile([128, 2, TILE], FP8, tag="d1hl")
                for cs in windows(tn, nsplit):
                    dr_win(ps, w1_t, w2_t, a2[t][0], a2[t][1], PD, 2, cs)
                    act_AD(ps, hh, hl, cs, od1, drain=(t >= nt - 2))
                d1a[t] = (hh, hl)

            def emit_d2(t, nsplit=1):
                off, tn, ws = tiles[t]
                w1_t, w2_t = wsets[ws][2], wsets[ws][3]
                od2 = wsets[ws][7]
                ps = ps_dec.tile([128, 2, TILE], F32, tag="pd", name="psC")
                hh = dp.tile([128, 2, TILE], FP8, tag="d2hh")
                hl = dp.tile([128, 2, TILE], FP8, tag="d2hl")
                for cs in windows(tn, nsplit):
                    dr_win(ps, w1_t, w2_t, d1a[t][0], d1a[t][1], PD, 2, cs)
                    act_AD(ps, hh, hl, cs, od2, drain=(t >= nt - 2))
                d2a[t] = (hh, hl)

            osbw = [None]

            def emit_d3(t, nsplit=1):
                off, tn, ws = tiles[t]
                w1_t, w2_t = wsets[ws][4], wsets[ws][5]
                od3 = wsets[ws][8]
                ps = ps_dec.tile([128, TILE], F32, tag="pd", name="psE")
                if tail_w and t == nt - 2:
                    osb = op.tile([128, TILE + tail_w], FP16, tag="osbw",
                                  bufs=1)
                    osbw[0] = osb
                elif tail_w and t == nt - 1:
                    osb = osbw[0]
                else:
                    osb = op.tile([128, TILE], FP16, tag="osb")
                for cs in windows(tn, nsplit):
                    cnt = 0
                    for x_t, w_t in ((d2a[t][0], w1_t), (d2a[t][0], w2_t),
                                     (d2a[t][1], w1_t)):
                        nc.tensor.matmul(ps[:, cs], w_t[:, :, :],
                                         x_t[:, :, cs],
                                         start=(cnt == 0), stop=(cnt == 2),
                                         perf_mode=DR)
                        cnt += 1
                    if tail_w and t == nt - 1:
                        # tail copy-out on the (idle) DVE
                        osl = slice(TILE + cs.start, TILE + cs.stop)
                        if with_bias:
                            nc.vector.tensor_scalar(osb[:, osl], ps[:, cs],
                                                    U,
                                                    bias_t[:, od3:od3 + 1],
                                                    Alu.mult, Alu.add)
                        else:
                            nc.vector.tensor_scalar(osb[:, osl], ps[:, cs],
                                                    U, None, Alu.mult)
                    elif with_bias:
                        nc.scalar.activation(osb[:, cs], ps[:, cs],
                                             ActF.Identity,
                                             bias=bias_t[:, od3:od3 + 1],
                                             scale=U)
                    else:
                        nc.scalar.activation(osb[:, cs], ps[:, cs],
                                             ActF.Identity, scale=U)
                if tail_w and t == nt - 1:
                    off7 = tiles[nt - 2][0]
                    nc.sync.dma_start(out[:, off7:off7 + TILE + tn],
                                      osb[:, :TILE + tn])
                elif not (tail_w and t == nt - 2):
                    nc.sync.dma_start(out[:, off:off + tn], osb[:, :tn])

            # ---- software pipeline: per step t
            # [L1(t)][d3(t-3)][d2(t-2)][L2(t)][d1(t-1)], with the last big
            # tile's d1 pulled into its own step (lag-0, fill-covered wait)
            # so the drain ladder starts two steps early.
            ft_cur[0] = (ft0h, ft0l)
            d1_done = set()
            for t in range(nt):
                emit_L1(t)
                if t == 0:
                    nc.sync.dma_start(we2h_t[:], we2h[:])
                    nc.sync.dma_start(we2l_t[:], we2l[:])
                    nc.sync.dma_start(wd1ah_t[:], wd1ah[:])
                    nc.sync.dma_start(wd1al_t[:], wd1al[:])
                if t + 1 < nt:
                    ft_cur[t + 1] = ftile(t + 1)
                if t == 0:
                    nc.sync.dma_start(wd2ah_t[:], wd2ah[:])
                    nc.sync.dma_start(wd2al_t[:], wd2al[:])
                    nc.sync.dma_start(wd3ah_t[:], wd3ah[:])
                    nc.sync.dma_start(wd3al_t[:], wd3al[:])
                elif t == 1 and tail_w:
                    nc.sync.dma_start(wd1bh_t[:], wd1bh[:])
                    nc.sync.dma_start(wd1bl_t[:], wd1bl[:])
                    nc.sync.dma_start(wd2bh_t[:], wd2bh[:])
                    nc.sync.dma_start(wd2bl_t[:], wd2bl[:])
                    nc.sync.dma_start(wd3bh_t[:], wd3bh[:])
                    nc.sync.dma_start(wd3bl_t[:], wd3bl[:])
                if 0 <= t - 3:
                    emit_d3(t - 3)
                if 0 <= t - 2:
                    emit_d2(t - 2)
                if t == 0:
                    fill(fills[0])
                elif t == 1:
                    fill(fills[1])
                emit_L2_p1(t)
                if 0 <= t - 1 and t - 1 not in d1_done:
                    if t == 1:
                        fill(fills[2])
                    emit_d1(t - 1)
                    d1_done.add(t - 1)
                emit_L2_p2(t)
                if t == nt - 2 and t not in d1_done:
                    emit_d1(t, pre=fills[3])
                    d1_done.add(t)
            # ---- drain
            emit_d2(nt - 2)
            if nt - 1 not in d1_done:
                emit_d1(nt - 1)
            emit_d3(nt - 3)
            fill(fills[4])
            emit_d2(nt - 1)
            emit_d3(nt - 2)
            fill(fills[5])
            emit_d3(nt - 1)

    nc.finalize()
    return nc


def _plan_tails(counts):
    """Assign overflow rows (beyond MAIN per category) to per-core tail
    slots: one category per core tail, tail_w rows max per core."""
    ov = {k: int(c) - MAIN for k, c in enumerate(counts) if c > MAIN}
    if not ov:
        return 0, [None] * N_CORES
    for tail_w in (32, 64, 128, 256, 384, 512):
        if sum(-(-v // tail_w) for v in ov.values()) <= N_CORES:
            break
    else:
        return None, None
    assign = []  # (cat, n_rows) per used core
    for k, v in sorted(ov.items()):
        while v > 0:
            take = min(v, tail_w)
            assign.append((k, take))
            v -= take
    assign += [None] * (N_CORES - len(assign))
    return tail_w, assign


def _pow2(v):
    return float(2.0 ** np.round(np.log2(max(v, 1e-30))))


def _split8(x):
    hi = np.asarray(x, np.float32).astype(E4)
    lo = (np.asarray(x, np.float32) - hi.astype(np.float32)).astype(E4)
    return hi, lo


def _wpack(w):
    """(C_, M) -> [128, C_/128, M] SBUF layout for single-DMA loads."""
    w = np.asarray(w)
    n = w.shape[0] // 128
    return np.ascontiguousarray(w.reshape(n, 128, -1).transpose(1, 0, 2))


def _chunkcols(b):
    return np.asarray(b, np.float32).reshape(-1).reshape(-1, 128).T


def _scales(features, We1f, We2f, We3f, Wd1f, Wd2f, Wd3f):
    """Shared pow2 scale chain (identical across cores; the graph bakes
    s2/s4/U as immediates and is cached per scale-tuple)."""
    sig_f = float(np.asarray(features, np.float32)[::37].std())
    c1 = _pow2(0.5 / sig_f)
    z1 = _pow2(0.35 / float(We1f.std()))
    z2 = _pow2(0.35 / float(We2f.std()))
    Wfs = [We3f @ Wd1f[k] for k in range(K)]
    z3 = _pow2(0.35 / max(float(w.std()) for w in Wfs))
    z4 = _pow2(0.35 / max(float(Wd2f[k].std()) for k in range(K)))
    z5 = _pow2(0.35 / max(float(Wd3f[k].std()) for k in range(K)))
    S1 = c1 * sig_f * z1 * np.linalg.norm(We1f) / np.sqrt(H1)
    r2 = S1 / np.sqrt(2)
    S2 = r2 * z2 * np.linalg.norm(We2f) / np.sqrt(H2)
    s2 = _pow2(1.2 / (S2 / np.sqrt(2)))
    U = 1.0 / (c1 * z1 * z2 * s2 * z3 * z4 * z5)
    return c1, z1, z2, z3, z4, z5, s2, U


def _pack_inputs(features, We1, be1, We2, be2, We3, be3,
                 Wd1, bd1, Wd2, bd2, Wd3, bd3, cat_idx,
                 tail_w, tails, with_bias):
    features = np.asarray(features, np.float32)
    cat = np.asarray(cat_idx).astype(np.int64)
    order = np.argsort(cat, kind="stable")
    counts = np.bincount(cat, minlength=N_CORES)
    starts = np.zeros(N_CORES + 1, np.int64)
    np.cumsum(counts, out=starts[1:])
    cat_rows = [order[starts[k]:starts[k + 1]] for k in range(N_CORES)]

    We1f = np.asarray(We1, np.float32)
    We2f = np.asarray(We2, np.float32)
    We3f = np.asarray(We3, np.float32)
    be3f = np.asarray(be3, np.float32)
    Wd1f = np.asarray(Wd1, np.float32)
    bd1f = np.asarray(bd1, np.float32)
    Wd2f = np.asarray(Wd2, np.float32)
    bd2f = np.asarray(bd2, np.float32)
    Wd3f = np.asarray(Wd3, np.float32)
    bd3f = np.asarray(bd3, np.float32)

    c1, z1, z2, z3, z4, z5, s2, U = _scales(
        features, We1f, We2f, We3f, Wd1f, Wd2f, Wd3f)
    Wfs = [We3f @ Wd1f[k] for k in range(K)]

    # cumulative psum pre-scales per layer (for bias columns)
    Pm1 = c1 * z1
    Pm2 = Pm1 * z2
    Pm3 = Pm2 * s2 * z3
    Pm4 = Pm3 * z4
    Pm5 = Pm4 * z5

    we1h_v, we1l_v = (_wpack(w) for w in _split8(We1f * z1))
    we2h_v, we2l_v = (_wpack(w) for w in _split8(We2f * z2))

    def dec_weights(k):
        wf = Wfs[k]
        d1h, d1l = _split8(wf * z3)
        d2h, d2l = _split8(Wd2f[k] * z4)
        d3h, d3l = _split8(Wd3f[k] * z5)
        return tuple(_wpack(w) for w in (d1h, d1l, d2h, d2l, d3h, d3l))

    def dec_bias(k):
        b1 = np.asarray(Wd1f[k]).T @ be3f + bd1f[k]
        return (_chunkcols(b1 * Pm3), _chunkcols(bd2f[k] * Pm4),
                _chunkcols(bd3f[k] * Pm5))

    used = {k: MAIN for k in range(N_CORES)}
    maps, row_maps = [], []
    for j in range(N_CORES):
        main_rows = cat_rows[j][:MAIN]
        f = np.zeros((MAIN, C), np.float32)
        f[:len(main_rows)] = features[main_rows]
        fT = np.ascontiguousarray(f.T) * c1
        fh, fl = _split8(fT)
        tail_rows = np.empty((0,), np.int64)
        tcat = j
        if tail_w and tails[j] is not None:
            tcat, n = tails[j]
            tail_rows = cat_rows[tcat][used[tcat]:used[tcat] + n]
            used[tcat] += n
        da = dec_weights(j)
        m = {
            "fthi": fh, "ftlo": fl,
            "we1h": we1h_v, "we1l": we1l_v,
            "we2h": we2h_v, "we2l": we2l_v,
            "wd1ah": da[0], "wd1al": da[1],
            "wd2ah": da[2], "wd2al": da[3],
            "wd3ah": da[4], "wd3al": da[5],
        }
        if tail_w:
            ft = np.zeros((tail_w, C), np.float32)
            ft[:len(tail_rows)] = features[tail_rows]
            ftT = np.ascontiguousarray(ft.T) * c1
            th, tl = _split8(ftT)
            db = dec_weights(tcat)
            m["ftthi"], m["fttlo"] = th, tl
            m["wd1bh"], m["wd1bl"] = db[0], db[1]
            m["wd2bh"], m["wd2bl"] = db[2], db[3]
            m["wd3bh"], m["wd3bl"] = db[4], db[5]
        if with_bias:
            bias_all = np.zeros((128, NBIAS), np.float32)
            bias_all[:, OB1:OB1 + 4] = _chunkcols(np.asarray(be1) * Pm1)
            bias_all[:, OB1S:OB1S + 4] = bias_all[:, OB1:OB1 + 4]
            bias_all[:, OB2:OB2 + 2] = _chunkcols(np.asarray(be2) * Pm2)
            bias_all[:, OB2S:OB2S + 2] = bias_all[:, OB2:OB2 + 2] * s2
            ba = dec_bias(j)
            bias_all[:, OAD1:OAD1 + 2] = ba[0]
            bias_all[:, OAD1S:OAD1S + 2] = ba[0]
            bias_all[:, OAD2:OAD2 + 2] = ba[1]
            bias_all[:, OAD3:OAD3 + 1] = ba[2] * U
            if tail_w:
                bb = dec_bias(tcat)
                bias_all[:, OBD1:OBD1 + 2] = bb[0]
                bias_all[:, OBD1S:OBD1S + 2] = bb[0]
                bias_all[:, OBD2:OBD2 + 2] = bb[1]
                bias_all[:, OBD3:OBD3 + 1] = bb[2] * U
            m["bias_all"] = bias_all
        maps.append(m)
        row_maps.append((main_rows, tail_rows))
    return maps, row_maps


_NC_CACHE = {}
_LAST_KEY = None


def _get_nc(key=None):
    global _LAST_KEY
    if key is None:
        key = _LAST_KEY if _LAST_KEY is not None else (
            32, False, 0.0625, 2.0 ** -13)
    if key not in _NC_CACHE:
        _NC_CACHE[key] = _build_nc(*key)
    _LAST_KEY = key
    return _NC_CACHE[key]


def kernel(**inputs) -> np.ndarray:
    cat = np.asarray(inputs["cat_idx"]).astype(np.int64)
    counts = np.bincount(cat, minlength=K)
    tail_w, tails = _plan_tails(counts)
    assert tail_w is not None, "category distribution too skewed for tails"
    with_bias = any(
        np.any(np.asarray(inputs[k], np.float32))
        for k in ("be1", "be2", "be3", "bd1", "bd2", "bd3"))
    sc = _scales(np.asarray(inputs["features"], np.float32),
                 np.asarray(inputs["We1"], np.float32),
                 np.asarray(inputs["We2"], np.float32),
                 np.asarray(inputs["We3"], np.float32),
                 np.asarray(inputs["Wd1"], np.float32),
                 np.asarray(inputs["Wd2"], np.float32),
                 np.asarray(inputs["Wd3"], np.float32))
    nc = _get_nc((tail_w, with_bias, sc[6], sc[7]))
    maps, row_maps = _pack_inputs(**inputs, tail_w=tail_w, tails=tails,
                                  with_bias=with_bias)
    res = bass_utils.run_bass_kernel_spmd(nc, maps, core_ids=list(range(N_CORES)))
    latent = np.zeros((B, LAT), np.float32)
    for j, r in enumerate(res.results):
        main_rows, tail_rows = row_maps[j]
        o = np.asarray(r["out"]).astype(np.float32)
        latent[main_rows] = o[:, :len(main_rows)].T
        if len(tail_rows):
            latent[tail_rows] = o[:, MAIN:MAIN + len(tail_rows)].T
    return latent
